# revision 33
# baseline (speedup 1.0000x reference)
"""LlamaAttention (B=1,S=2048,D=2048,NH=32,NKV=8,HD=64) on 8 Trainium2 cores.

Strategy: head tensor-parallel. Core c owns q-heads 4c..4c+3 and kv-head c.
Everything on-device runs in a transposed layout ([feature, seq]) so all
matmuls contract over partitions; rope pairs are de-interleaved by permuting
weight rows on the host. Scores are built transposed (S.T[sk, sq]) so the
softmax denominator comes from an appended ones-column in the V matmul and
the attention@V step needs no on-chip transposes. All matmuls use float32r.
The additive mask is handled by exp(S+M)=exp(S)*exp(M): the host classifies
128x128 blocks of exp(M).T into ones/zero/other, the kernel skips zero
blocks, multiplies "other" blocks, and leaves "ones" blocks untouched.
"""
import sys
if '/opt/trn_rl_repo' not in sys.path:
    sys.path.insert(0, '/opt/trn_rl_repo')

import numpy as np

B, S, D = 1, 2048, 2048
NH, NKV, HD = 32, 8, 64
NCORES = 8
HPC = NH // NCORES          # 4 q heads per core
EPS = 1e-5
INV_SCALE = 1.0 / (float(HD) ** 0.5)
P = 128
NKT = D // P                # 16 contraction tiles for projections
CH = 512                    # sq chunk width
NCH = S // CH               # 4 chunks
NB = S // P                 # 16 sk blocks
SUB = CH // P               # 4 sq sub-cols per chunk

PERM = np.concatenate([np.arange(0, HD, 2), np.arange(1, HD, 2)])  # deinterleave


def _classify_mask(mask):
    """mask: [S, S] additive (mask[q, k]). Returns (grid, tiles, expmT).
    grid[b][j] for the [128,128] block of expmT rows 128b, cols 128j:
    -1 = all ones (no-op), -2 = all zeros (skip), >=0 = index into tiles."""
    expmT = np.exp(np.minimum(mask.T.astype(np.float64), 0.0)).astype(np.float32)
    grid = np.empty((NB, S // P), dtype=np.int64)
    tiles = []
    seen = {}
    for b in range(NB):
        for j in range(S // P):
            blk = expmT[b * P:(b + 1) * P, j * P:(j + 1) * P]
            mn, mx = blk.min(), blk.max()
            if mn == 1.0 and mx == 1.0:
                grid[b, j] = -1
            elif mx == 0.0:
                grid[b, j] = -2
            else:
                key = blk.tobytes()
                if key not in seen:
                    seen[key] = len(tiles)
                    tiles.append(blk)
                grid[b, j] = seen[key]
    return grid, tiles


def _build(grid, ntiles):
    """Trace + compile the SPMD kernel for one core given the mask block grid."""
    import concourse.bass as bass
    import concourse.tile as tile
    from concourse import bacc, mybir
    from concourse.masks import make_identity
    F32 = mybir.dt.float32
    F32R = mybir.dt.float32r
    Exp = mybir.ActivationFunctionType.Exp
    Sqrt = mybir.ActivationFunctionType.Sqrt

    nc = bacc.Bacc('TRN2', target_bir_lowering=False)
    xT_d = nc.dram_tensor('xT', [D, S], F32R, kind='ExternalInput')
    wqT_d = nc.dram_tensor('wqT', [D, HPC * HD], F32R, kind='ExternalInput')
    wkvT_d = nc.dram_tensor('wkvT', [D, 2 * HD], F32R, kind='ExternalInput')
    woT_d = nc.dram_tensor('woT', [HPC * HD, D], F32R, kind='ExternalInput')
    cq_d = nc.dram_tensor('cq', [P, S], F32, kind='ExternalInput')
    sq_d = nc.dram_tensor('sq', [P, S], F32, kind='ExternalInput')
    ck_d = nc.dram_tensor('ck', [HD, S], F32, kind='ExternalInput')
    sk_d = nc.dram_tensor('sk', [HD, S], F32, kind='ExternalInput')
    nt = max(ntiles, 1)
    em_d = nc.dram_tensor('em', [nt, P, P], F32, kind='ExternalInput')
    wrow_d = nc.dram_tensor('wrow', [2, 2 * P], F32R, kind='ExternalInput')
    yT_d = nc.dram_tensor('yT', [D, S], F32, kind='ExternalOutput')
    kh_d = nc.dram_tensor('kh', [HD, S], F32, kind='ExternalOutput')
    vh_d = nc.dram_tensor('vh', [S, HD], F32, kind='ExternalOutput')

    # per-chunk block schedule (shared by all heads): list of (b, lo_subcol)
    sched = []
    for c in range(NCH):
        blocks = []
        for b in range(NB):
            stats = [grid[b, SUB * c + j] for j in range(SUB)]
            if all(s == -2 for s in stats):
                continue
            j0 = min(j for j in range(SUB) if stats[j] != -2)
            blocks.append((b, j0, stats))
        assert blocks, 'fully masked chunk'
        # first block must cover the full chunk so PSUM cols all get written
        b0, _, st0 = blocks[0]
        blocks[0] = (b0, 0, st0)
        sched.append(blocks)

    with tile.TileContext(nc) as tc:
        with tc.tile_pool(name='cst', bufs=1) as cst, \
             tc.tile_pool(name='wts', bufs=1) as wts, \
             tc.tile_pool(name='per', bufs=1) as per:
            ident = cst.tile([P, P], F32)
            make_identity(nc, ident)
            ones_f = cst.tile([P, 65], F32)
            nc.vector.memset(ones_f[:], 1.0)
            ones_row = cst.tile([P, 64], F32R)
            nc.vector.tensor_copy(ones_row[:], ones_f[:, 0:64])
            ones_col = cst.tile([P, 1], F32R)
            nc.vector.tensor_copy(ones_col[:], ones_f[:, 0:1])
            # msq reduction patterns: col 0 sums rows 0:64, col 1 sums rows 64:128
            pat_f = cst.tile([P, 2], F32)
            nc.vector.memset(pat_f[:], 0.0)
            nc.vector.memset(pat_f[0:64, 0:1], 1.0)
            nc.vector.memset(pat_f[64:128, 1:2], 1.0)
            pat_q = cst.tile([P, 2], F32R)
            nc.vector.tensor_copy(pat_q[:], pat_f[:])
            eps_t = cst.tile([P, 1], F32)
            nc.vector.memset(eps_t[:], EPS)

            cq = cst.tile([P, S], F32)
            nc.sync.dma_start(cq[:], cq_d[:])
            sq = cst.tile([P, S], F32)
            nc.sync.dma_start(sq[:], sq_d[:])
            ck = cst.tile([HD, S], F32)
            nc.sync.dma_start(ck[:], ck_d[:])
            sk = cst.tile([HD, S], F32)
            nc.sync.dma_start(sk[:], sk_d[:])
            em = cst.tile([P, nt, P], F32)
            nc.sync.dma_start(em[:], em_d.rearrange('u p j -> p u j'))
            wrow = cst.tile([2, 2 * P], F32R)
            nc.sync.dma_start(wrow[:], wrow_d[:])

            wq = wts.tile([P, NKT, HPC * HD], F32R)
            nc.sync.dma_start(wq[:], wqT_d.rearrange('(o p) m -> p o m', p=P))
            wkv = wts.tile([P, NKT, 2 * HD], F32R)
            nc.sync.dma_start(wkv[:], wkvT_d.rearrange('(o p) m -> p o m', p=P))

            qtn = per.tile([P, 2, S], F32R)       # normalized rope'd Q.T
            kst = per.tile([P, S], F32R)          # K.T normed, duplicated rows
            vaug = per.tile([P, NB, 65], F32R)    # V natural + ones col
            ot = per.tile([P, 2, S], F32R)        # attention out (transposed)

            xT_r = xT_d.rearrange('(o p) s -> p o s', p=P)

            # ---------------- phase 1: projections + rope + rmsnorm ----------
            with tc.tile_pool(name='xp', bufs=6) as xp, \
                 tc.tile_pool(name='t1', bufs=2) as t1, \
                 tc.tile_pool(name='pp', bufs=3, space='PSUM') as pp, \
                 tc.tile_pool(name='pr', bufs=1, space='PSUM') as pr, \
                 tc.tile_pool(name='pu', bufs=2, space='PSUM') as pu:
                for c in range(NCH):
                    cs = slice(c * CH, (c + 1) * CH)
                    pss = [pp.tile([P, CH], F32, tag='proj', name=f'proj{i}')
                           for i in range(3)]
                    for kt in range(NKT):
                        xt = xp.tile([P, CH], F32R, tag='x')
                        nc.sync.dma_start(xt[:], xT_r[:, kt, cs])
                        nc.tensor.matmul(pss[0][:], wq[:, kt, 0:P], xt[:],
                                         start=(kt == 0), stop=(kt == NKT - 1))
                        nc.tensor.matmul(pss[1][:], wq[:, kt, P:2 * P], xt[:],
                                         start=(kt == 0), stop=(kt == NKT - 1))
                        nc.tensor.matmul(pss[2][:], wkv[:, kt, :], xt[:],
                                         start=(kt == 0), stop=(kt == NKT - 1))
                    for blk in range(3):   # 0,1: q pairs; 2: kv
                        ps = pss[blk]
                        raw = t1.tile([P, CH], F32, tag='raw')
                        nc.scalar.copy(raw[:], ps[:])

                        if blk < 2:
                            # rope on 2 heads: slabs of 32, swap re<->im via PSUM
                            ro = t1.tile([P, CH], F32, tag='ro')
                            nc.vector.tensor_mul(ro[:], raw[:], cq[:, cs])
                            u = pu.tile([P, CH], F32, tag='u')
                            nc.vector.tensor_mul(u[:], raw[:], sq[:, cs])
                            for sl in range(4):
                                a, bsl = sl * 32, (sl ^ 1) * 32
                                nc.vector.tensor_add(
                                    ro[a:a + 32], ro[a:a + 32], u[bsl:bsl + 32])
                            # msq over each head's 64 rows
                            sqr = t1.tile([P, CH], F32R, tag='sqr')
                            nc.vector.tensor_mul(sqr[:], ro[:], ro[:])
                            msq = pr.tile([2, CH], F32, tag='msq')
                            nc.tensor.matmul(msq[:], pat_q[:], sqr[:],
                                             start=True, stop=True)
                            rs = t1.tile([2, CH], F32, tag='rs')
                            rr = t1.tile([2, CH], F32R, tag='rr')
                            nc.scalar.activation(rs[:], msq[:], func=Sqrt,
                                                 bias=eps_t[0:2], scale=1.0 / HD)
                            with nc.allow_low_precision(reason='f32r recip'):
                                nc.vector.reciprocal(rr[:], rs[:])
                            rb = pr.tile([P, CH], F32, tag='rb')
                            nc.tensor.matmul(rb[:], wrow[:, 0:P], rr[:],
                                             start=True, stop=True)
                            nc.vector.tensor_mul(qtn[:, blk, cs], ro[:], rb[:])
                        else:
                            # k rows 0:64 -> rope+norm+dup; v rows 64:128 -> transpose
                            ro = t1.tile([HD, CH], F32, tag='rok')
                            nc.vector.tensor_mul(ro[:], raw[0:HD], ck[:, cs])
                            u = pu.tile([P, CH], F32, tag='u')
                            nc.vector.tensor_mul(u[0:HD], raw[0:HD], sk[:, cs])
                            for sl in range(2):
                                a, bsl = sl * 32, (sl ^ 1) * 32
                                nc.vector.tensor_add(
                                    ro[a:a + 32], ro[a:a + 32], u[bsl:bsl + 32])
                            sqr = t1.tile([HD, CH], F32R, tag='sqk')
                            nc.vector.tensor_mul(sqr[:], ro[:], ro[:])
                            msq = pr.tile([1, CH], F32, tag='msq')
                            nc.tensor.matmul(msq[:], pat_q[0:64, 0:1], sqr[:],
                                             start=True, stop=True)
                            rs = t1.tile([1, CH], F32, tag='rsk')
                            rr = t1.tile([1, CH], F32R, tag='rrk')
                            nc.scalar.activation(rs[:], msq[:], func=Sqrt,
                                                 bias=eps_t[0:1], scale=1.0 / HD)
                            with nc.allow_low_precision(reason='f32r recip'):
                                nc.vector.reciprocal(rr[:], rs[:])
                            rb = pr.tile([64, CH], F32, tag='rb')
                            nc.tensor.matmul(rb[:], wrow[0:1, P:P + 64], rr[:],
                                             start=True, stop=True)
                            nc.vector.tensor_mul(kst[0:HD, cs], ro[:], rb[:])
                            nc.vector.tensor_mul(kst[HD:P, cs], ro[:], rb[:])
                            for t in range(SUB):
                                pt = pu.tile([P, HD], F32, tag='u')
                                nc.tensor.transpose(
                                    pt[:], raw[HD:P, t * P:(t + 1) * P],
                                    ident[HD:P, HD:P])
                                vb = c * SUB + t
                                nc.scalar.copy(vaug[:, vb, 0:HD], pt[:])
                                nc.vector.tensor_copy(vaug[:, vb, HD:HD + 1],
                                                      ones_col[:])

            # ---------------- phase 2: attention ----------------------------
            with tc.tile_pool(name='es', bufs=4) as es, \
                 tc.tile_pool(name='t3', bufs=4) as t3, \
                 tc.tile_pool(name='sp', bufs=3, space='PSUM') as sp, \
                 tc.tile_pool(name='op', bufs=2, space='PSUM') as op, \
                 tc.tile_pool(name='zp', bufs=2, space='PSUM') as zp:
                for h in range(HPC):
                    qb, base = h // 2, 64 * (h % 2)
                    rows = slice(base, base + HD)
                    for c in range(NCH):
                        cs = slice(c * CH, (c + 1) * CH)
                        blocks = sched[c]
                        oa = op.tile([65, CH], F32, tag='oa')
                        for bi, (b, j0, stats) in enumerate(blocks):
                            lo = j0 * P
                            n = CH - lo
                            s_ps = sp.tile([P, CH], F32, tag='s')
                            nc.tensor.matmul(
                                s_ps[:, :n], kst[rows, b * P:(b + 1) * P],
                                qtn[rows, qb, c * CH + lo:(c + 1) * CH],
                                start=True, stop=True)
                            e = es.tile([P, CH], F32R, tag='e')
                            nc.scalar.activation(e[:, :n], s_ps[:, :n], func=Exp,
                                                 scale=INV_SCALE)
                            for j in range(j0, SUB):
                                g = stats[j]
                                if g == -1:
                                    continue
                                jc = slice(j * P - lo, (j + 1) * P - lo)
                                if g == -2:
                                    nc.vector.tensor_scalar_mul(e[:, jc], e[:, jc], 0.0)
                                else:
                                    nc.vector.tensor_mul(e[:, jc], e[:, jc],
                                                         em[:, g, :])
                            nc.tensor.matmul(oa[:, lo:], vaug[:, b, :], e[:, :n],
                                             start=(bi == 0),
                                             stop=(bi == len(blocks) - 1))
                        rcp = t3.tile([65, CH], F32R, tag='rcp')
                        with nc.allow_low_precision(reason='f32r recip'):
                            nc.vector.reciprocal(rcp[64:65, :], oa[64:65, :])
                        zb = zp.tile([64, CH], F32, tag='zb')
                        nc.tensor.matmul(zb[:], ones_row[64:65, :], rcp[64:65, :],
                                         start=True, stop=True,
                                         tile_position=(64, 0))
                        zs = t3.tile([64, CH], F32, tag='zs')
                        nc.scalar.copy(zs[:], zb[:])
                        nc.vector.tensor_mul(ot[rows, qb, cs], oa[0:HD, :], zs[:])

            # ---------------- phase 3: output projection ---------------------
            yT_r = yT_d.rearrange('(o p) s -> p o s', p=P)
            with tc.tile_pool(name='t4', bufs=4) as t4, \
                 tc.tile_pool(name='w3', bufs=1) as w3, \
                 tc.tile_pool(name='yp', bufs=4, space='PSUM') as yp:
                wo = w3.tile([P, 2, S], F32R)
                nc.sync.dma_start(wo[:], woT_d.rearrange('(o p) m -> p o m', p=P))
                for blk in range(NKT):
                    for c in range(NCH):
                        cs = slice(c * CH, (c + 1) * CH)
                        ps = yp.tile([P, CH], F32, tag='y')
                        for kt in range(2):
                            nc.tensor.matmul(
                                ps[:], wo[:, kt, blk * P:(blk + 1) * P],
                                ot[:, kt, cs], start=(kt == 0), stop=(kt == 1))
                        ys = t4.tile([P, CH], F32, tag='ys')
                        nc.scalar.copy(ys[:], ps[:])
                        nc.sync.dma_start(yT_r[:, blk, cs], ys[:])
                nc.sync.dma_start(kh_d[:], kst.bitcast(F32)[0:HD, :])
                nc.sync.dma_start(
                    vh_d.rearrange('(o p) h -> p o h', p=P),
                    vaug.bitcast(F32)[:, :, 0:HD])
    nc.compile()
    return nc


_cache = {}


def kernel(hidden_states, freqs_cos, freqs_sin, atten_mask, wq, wk, wv, wo,
           q_norm_w, k_norm_w):
    from concourse.bass_utils import run_bass_kernel_spmd

    hs = np.asarray(hidden_states, dtype=np.float32)
    fc = np.asarray(freqs_cos, dtype=np.float32)
    fs = np.asarray(freqs_sin, dtype=np.float32)
    am = np.asarray(atten_mask, dtype=np.float32)
    wq = np.asarray(wq, dtype=np.float32)
    wk = np.asarray(wk, dtype=np.float32)
    wv = np.asarray(wv, dtype=np.float32)
    wo = np.asarray(wo, dtype=np.float32)
    qw = np.asarray(q_norm_w, dtype=np.float32)
    kw = np.asarray(k_norm_w, dtype=np.float32)

    grid, tiles = _classify_mask(am[0, 0])
    key = (grid.tobytes(), len(tiles))
    if key not in _cache:
        _cache[key] = _build(grid, len(tiles))
    nc = _cache[key]

    em = np.stack(tiles) if tiles else np.zeros((1, P, P), np.float32)
    xT = np.ascontiguousarray(hs[0].T)

    # rope tables in transposed + deinterleaved layout
    cosT = np.ascontiguousarray(fc.T)  # [32, S]
    sinT = np.ascontiguousarray(fs.T)
    qw_p = qw[PERM]
    kw_p = kw[PERM]

    c64 = np.concatenate([cosT, cosT], 0)           # [64, S]
    s64 = np.concatenate([sinT, -sinT], 0)          # indexed by SOURCE row
    cq_t, sq_t = np.tile(c64, (2, 1)), np.tile(s64, (2, 1))
    ck_t, sk_t = c64, s64

    # norm weights as lhsT for the rsqrt-broadcast matmuls:
    # q: lhsT = wrow[0:2, 0:128] (K=2: row 0 scales head-even rows, row 1 odd)
    # k: lhsT = wrow[0:1, 128:192]
    wrow = np.zeros((2, 2 * P), dtype=np.float32)
    wrow[0, 0:64] = qw_p
    wrow[1, 64:128] = qw_p
    wrow[0, P:P + 64] = kw_p

    wq4 = wq.reshape(NH, HD, D)
    wk8 = wk.reshape(NKV, HD, D)
    wv8 = wv.reshape(NKV, HD, D)

    in_maps = []
    for c in range(NCORES):
        wq_c = wq4[HPC * c:HPC * (c + 1)][:, PERM, :]      # [4, 64, D]
        wqT = np.ascontiguousarray(
            wq_c.reshape(HPC * HD, D).T)                    # [D, 256]
        wkv = np.concatenate([wk8[c][PERM], wv8[c]], 0)     # [128, D]
        wkvT = np.ascontiguousarray(wkv.T)
        woT = np.ascontiguousarray(wo[:, HPC * HD * c:HPC * HD * (c + 1)].T)
        in_maps.append(dict(xT=xT, wqT=wqT, wkvT=wkvT, woT=woT,
                            cq=np.ascontiguousarray(cq_t),
                            sq=np.ascontiguousarray(sq_t),
                            ck=np.ascontiguousarray(ck_t),
                            sk=np.ascontiguousarray(sk_t), em=em, wrow=wrow))

    global _last_in_maps
    _last_in_maps = in_maps
    res = run_bass_kernel_spmd(nc, in_maps, core_ids=list(range(NCORES)))
    outs = res.results

    y = np.zeros((S, D), dtype=np.float32)
    khT = np.empty((B, NH, HD, S), dtype=np.float32)
    vh = np.empty((B, NH, S, HD), dtype=np.float32)
    inv = np.argsort(PERM)
    for c in range(NCORES):
        y += outs[c]['yT'].T
        k_nat = outs[c]['kh'][inv]          # un-permute rows
        for h in range(HPC):
            khT[0, HPC * c + h] = k_nat
            vh[0, HPC * c + h] = outs[c]['vh']
    return y.reshape(B, S, D), khT, vh


# revision 42
# speedup vs baseline: 1.0020x; 1.0020x over previous
"""LlamaAttention (B=1,S=2048,D=2048,NH=32,NKV=8,HD=64) on 8 Trainium2 cores.

Strategy: head tensor-parallel. Core c owns q-heads 4c..4c+3 and kv-head c.
Everything on-device runs in a transposed layout ([feature, seq]) so all
matmuls contract over partitions; rope pairs are de-interleaved by permuting
weight rows on the host. Scores are built transposed (S.T[sk, sq]) so the
softmax denominator comes from an appended ones-column in the V matmul and
the attention@V step needs no on-chip transposes. All matmuls use float32r.
The additive mask is handled by exp(S+M)=exp(S)*exp(M): the host classifies
128x128 blocks of exp(M).T into ones/zero/other, the kernel skips zero
blocks, multiplies "other" blocks, and leaves "ones" blocks untouched.
"""
import sys
if '/opt/trn_rl_repo' not in sys.path:
    sys.path.insert(0, '/opt/trn_rl_repo')

import numpy as np

B, S, D = 1, 2048, 2048
NH, NKV, HD = 32, 8, 64
NCORES = 8
HPC = NH // NCORES          # 4 q heads per core
EPS = 1e-5
INV_SCALE = 1.0 / (float(HD) ** 0.5)
P = 128
NKT = D // P                # 16 contraction tiles for projections
CH = 512                    # sq chunk width
NCH = S // CH               # 4 chunks
NB = S // P                 # 16 sk blocks
SUB = CH // P               # 4 sq sub-cols per chunk

PERM = np.concatenate([np.arange(0, HD, 2), np.arange(1, HD, 2)])  # deinterleave


def _classify_mask(mask):
    """mask: [S, S] additive (mask[q, k]). Returns (grid, tiles, expmT).
    grid[b][j] for the [128,128] block of expmT rows 128b, cols 128j:
    -1 = all ones (no-op), -2 = all zeros (skip), >=0 = index into tiles."""
    expmT = np.exp(np.minimum(mask.T.astype(np.float64), 0.0)).astype(np.float32)
    grid = np.empty((NB, S // P), dtype=np.int64)
    tiles = []
    seen = {}
    for b in range(NB):
        for j in range(S // P):
            blk = expmT[b * P:(b + 1) * P, j * P:(j + 1) * P]
            mn, mx = blk.min(), blk.max()
            if mn == 1.0 and mx == 1.0:
                grid[b, j] = -1
            elif mx == 0.0:
                grid[b, j] = -2
            else:
                key = blk.tobytes()
                if key not in seen:
                    seen[key] = len(tiles)
                    tiles.append(blk)
                grid[b, j] = seen[key]
    return grid, tiles


def _build(grid, ntiles):
    """Trace + compile the SPMD kernel for one core given the mask block grid."""
    import concourse.bass as bass
    import concourse.tile as tile
    from concourse import bacc, mybir
    from concourse.masks import make_identity
    F32 = mybir.dt.float32
    F32R = mybir.dt.float32r
    Exp = mybir.ActivationFunctionType.Exp
    Ln = mybir.ActivationFunctionType.Ln

    nc = bacc.Bacc('TRN2', target_bir_lowering=False)
    xT_d = nc.dram_tensor('xT', [D, S], F32R, kind='ExternalInput')
    wqT_d = nc.dram_tensor('wqT', [D, HPC * HD], F32R, kind='ExternalInput')
    wkvT_d = nc.dram_tensor('wkvT', [D, 2 * HD], F32R, kind='ExternalInput')
    woT_d = nc.dram_tensor('woT', [HPC * HD, D], F32R, kind='ExternalInput')
    cq_d = nc.dram_tensor('cq', [P, S], F32, kind='ExternalInput')
    sq_d = nc.dram_tensor('sq', [P, S], F32, kind='ExternalInput')
    ck_d = nc.dram_tensor('ck', [HD, S], F32, kind='ExternalInput')
    sk_d = nc.dram_tensor('sk', [HD, S], F32, kind='ExternalInput')
    nt = max(ntiles, 1)
    em_d = nc.dram_tensor('em', [nt, P, P], F32, kind='ExternalInput')
    wrow_d = nc.dram_tensor('wrow', [2, 2 * P], F32R, kind='ExternalInput')
    yT_d = nc.dram_tensor('yT', [D, S], F32, kind='ExternalOutput')
    kh_d = nc.dram_tensor('kh', [HD, S], F32, kind='ExternalOutput')
    vh_d = nc.dram_tensor('vh', [S, HD], F32, kind='ExternalOutput')

    # per-chunk block schedule (shared by all heads): list of (b, lo_subcol)
    sched = []
    for c in range(NCH):
        blocks = []
        for b in range(NB):
            stats = [grid[b, SUB * c + j] for j in range(SUB)]
            if all(s == -2 for s in stats):
                continue
            j0 = min(j for j in range(SUB) if stats[j] != -2)
            blocks.append((b, j0, stats))
        assert blocks, 'fully masked chunk'
        # first block must cover the full chunk so PSUM cols all get written
        b0, _, st0 = blocks[0]
        blocks[0] = (b0, 0, st0)
        sched.append(blocks)

    with tile.TileContext(nc) as tc:
        with tc.tile_pool(name='cst', bufs=1) as cst, \
             tc.tile_pool(name='wts', bufs=1) as wts, \
             tc.tile_pool(name='per', bufs=1) as per:
            ident = cst.tile([P, P], F32)
            make_identity(nc, ident)
            ones_f = cst.tile([P, 65], F32)
            nc.vector.memset(ones_f[:], 1.0)
            ones_row = cst.tile([P, 64], F32R)
            nc.vector.tensor_copy(ones_row[:], ones_f[:, 0:64])
            ones_col = cst.tile([P, 1], F32R)
            nc.vector.tensor_copy(ones_col[:], ones_f[:, 0:1])
            # msq reduction patterns: col 0 sums rows 0:64, col 1 sums rows 64:128
            pat_f = cst.tile([P, 2], F32)
            nc.vector.memset(pat_f[:], 0.0)
            nc.vector.memset(pat_f[0:64, 0:1], 1.0)
            nc.vector.memset(pat_f[64:128, 1:2], 1.0)
            pat_q = cst.tile([P, 2], F32R)
            nc.vector.tensor_copy(pat_q[:], pat_f[:])
            eps_t = cst.tile([P, 1], F32)
            nc.vector.memset(eps_t[:], EPS)

            cq = cst.tile([P, S], F32)
            nc.sync.dma_start(cq[:], cq_d[:])
            sq = cst.tile([P, S], F32)
            nc.sync.dma_start(sq[:], sq_d[:])
            ck = cst.tile([HD, S], F32)
            nc.sync.dma_start(ck[:], ck_d[:])
            sk = cst.tile([HD, S], F32)
            nc.sync.dma_start(sk[:], sk_d[:])
            em = cst.tile([P, nt, P], F32)
            nc.sync.dma_start(em[:], em_d.rearrange('u p j -> p u j'))
            wrow = cst.tile([2, 2 * P], F32R)
            nc.sync.dma_start(wrow[:], wrow_d[:])

            wq = wts.tile([P, NKT, HPC * HD], F32R)
            nc.sync.dma_start(wq[:], wqT_d.rearrange('(o p) m -> p o m', p=P))
            wkv = wts.tile([P, NKT, 2 * HD], F32R)
            nc.sync.dma_start(wkv[:], wkvT_d.rearrange('(o p) m -> p o m', p=P))

            qtn = per.tile([P, 2, S], F32R)       # normalized rope'd Q.T
            kst = per.tile([P, S], F32R)          # K.T normed, duplicated rows
            vaug = per.tile([P, NB, 65], F32R)    # V natural + ones col
            ot = per.tile([P, 2, S], F32R)        # attention out (transposed)
            nc.vector.tensor_copy(vaug[:, :, HD:HD + 1], ones_f[:, 0:NB])

            xT_r = xT_d.rearrange('(o p) s -> p o s', p=P)

            # ---------------- phase 1: projections + rope + rmsnorm ----------
            with tc.tile_pool(name='xp', bufs=6) as xp, \
                 tc.tile_pool(name='t1', bufs=2) as t1, \
                 tc.tile_pool(name='pp', bufs=3, space='PSUM') as pp, \
                 tc.tile_pool(name='pr', bufs=1, space='PSUM') as pr, \
                 tc.tile_pool(name='pu', bufs=2, space='PSUM') as pu:
                for c in range(NCH):
                    cs = slice(c * CH, (c + 1) * CH)
                    pss = [pp.tile([P, CH], F32, tag='proj', name=f'proj{i}')
                           for i in range(3)]
                    for kt in range(NKT):
                        xt = xp.tile([P, CH], F32R, tag='x')
                        nc.sync.dma_start(xt[:], xT_r[:, kt, cs])
                        nc.tensor.matmul(pss[0][:], wq[:, kt, 0:P], xt[:],
                                         start=(kt == 0), stop=(kt == NKT - 1))
                        nc.tensor.matmul(pss[1][:], wq[:, kt, P:2 * P], xt[:],
                                         start=(kt == 0), stop=(kt == NKT - 1))
                        nc.tensor.matmul(pss[2][:], wkv[:, kt, :], xt[:],
                                         start=(kt == 0), stop=(kt == NKT - 1))
                    for blk in range(3):   # 0,1: q pairs; 2: kv
                        ps = pss[blk]
                        raw = t1.tile([P, CH], F32, tag='raw')
                        nc.any.tensor_copy(raw[:], ps[:])

                        if blk < 2:
                            # rope on 2 heads: slabs of 32, swap re<->im via PSUM
                            ro = t1.tile([P, CH], F32, tag='ro')
                            nc.vector.tensor_mul(ro[:], raw[:], cq[:, cs])
                            u = pu.tile([P, CH], F32, tag='u')
                            nc.vector.tensor_mul(u[:], raw[:], sq[:, cs])
                            for sl in range(4):
                                a, bsl = sl * 32, (sl ^ 1) * 32
                                nc.vector.tensor_add(
                                    ro[a:a + 32], ro[a:a + 32], u[bsl:bsl + 32])
                            # msq over each head's 64 rows
                            sqr = t1.tile([P, CH], F32R, tag='sqr')
                            nc.vector.tensor_mul(sqr[:], ro[:], ro[:])
                            msq = pr.tile([2, CH], F32, tag='msq')
                            nc.tensor.matmul(msq[:], pat_q[:], sqr[:],
                                             start=True, stop=True)
                            rs = t1.tile([2, CH], F32, tag='rs')
                            rr = t1.tile([2, CH], F32R, tag='rr')
                            nc.scalar.activation(rs[:], msq[:], func=Ln,
                                                 bias=eps_t[0:2], scale=1.0 / HD)
                            nc.scalar.activation(rr[:], rs[:], func=Exp,
                                                 scale=-0.5)
                            rb = pr.tile([P, CH], F32, tag='rb')
                            nc.tensor.matmul(rb[:], wrow[:, 0:P], rr[:],
                                             start=True, stop=True)
                            nc.vector.tensor_mul(qtn[:, blk, cs], ro[:], rb[:])
                        else:
                            # k rows 0:64 -> rope+norm+dup; v rows 64:128 -> transpose
                            ro = t1.tile([HD, CH], F32, tag='rok')
                            nc.vector.tensor_mul(ro[:], raw[0:HD], ck[:, cs])
                            u = pu.tile([P, CH], F32, tag='u')
                            nc.vector.tensor_mul(u[0:HD], raw[0:HD], sk[:, cs])
                            for sl in range(2):
                                a, bsl = sl * 32, (sl ^ 1) * 32
                                nc.vector.tensor_add(
                                    ro[a:a + 32], ro[a:a + 32], u[bsl:bsl + 32])
                            sqr = t1.tile([HD, CH], F32R, tag='sqk')
                            nc.vector.tensor_mul(sqr[:], ro[:], ro[:])
                            msq = pr.tile([1, CH], F32, tag='msq')
                            nc.tensor.matmul(msq[:], pat_q[0:64, 0:1], sqr[:],
                                             start=True, stop=True)
                            rs = t1.tile([1, CH], F32, tag='rsk')
                            rr = t1.tile([1, CH], F32R, tag='rrk')
                            nc.scalar.activation(rs[:], msq[:], func=Ln,
                                                 bias=eps_t[0:1], scale=1.0 / HD)
                            nc.scalar.activation(rr[:], rs[:], func=Exp,
                                                 scale=-0.5)
                            rb = pr.tile([64, CH], F32, tag='rb')
                            nc.tensor.matmul(rb[:], wrow[0:1, P:P + 64], rr[:],
                                             start=True, stop=True)
                            nc.vector.tensor_mul(kst[0:HD, cs], ro[:], rb[:])
                            nc.vector.tensor_mul(kst[HD:P, cs], ro[:], rb[:])
                            for t in range(SUB):
                                pt = pu.tile([P, HD], F32, tag='u')
                                nc.tensor.transpose(
                                    pt[:], raw[HD:P, t * P:(t + 1) * P],
                                    ident[HD:P, HD:P])
                                vb = c * SUB + t
                                nc.any.tensor_copy(vaug[:, vb, 0:HD], pt[:])

            # ---------------- phase 2: attention ----------------------------
            # head pairs share a q-block; their S-matmuls use PE row groups
            # (0,0)/(64,0) and run concurrently on the array.
            with tc.tile_pool(name='es', bufs=4) as es, \
                 tc.tile_pool(name='t3', bufs=3) as t3, \
                 tc.tile_pool(name='sp', bufs=3, space='PSUM') as sp, \
                 tc.tile_pool(name='op', bufs=3, space='PSUM') as op, \
                 tc.tile_pool(name='zp', bufs=1, space='PSUM') as zp:
                for hp in range(2):
                    for c in range(NCH):
                        cs = slice(c * CH, (c + 1) * CH)
                        blocks = sched[c]
                        oas = [op.tile([65, CH], F32, tag='oa', name=f'oa{hh}')
                               for hh in range(2)]
                        for bi, (b, j0, stats) in enumerate(blocks):
                            lo = j0 * P
                            n = CH - lo
                            eps_pair = []
                            for hh in range(2):
                                rows = slice(64 * hh, 64 * hh + HD)
                                s_ps = sp.tile([P, CH], F32, tag='s',
                                               name=f's{hh}')
                                nc.tensor.matmul(
                                    s_ps[:, :n], kst[rows, b * P:(b + 1) * P],
                                    qtn[rows, hp, c * CH + lo:(c + 1) * CH],
                                    start=True, stop=True)
                                e = es.tile([P, CH], F32R, tag='e',
                                            name=f'e{hh}')
                                nc.scalar.activation(e[:, :n], s_ps[:, :n],
                                                     func=Exp, scale=INV_SCALE)
                                for j in range(j0, SUB):
                                    g = stats[j]
                                    if g == -1:
                                        continue
                                    jc = slice(j * P - lo, (j + 1) * P - lo)
                                    if g == -2:
                                        nc.vector.tensor_scalar_mul(
                                            e[:, jc], e[:, jc], 0.0)
                                    else:
                                        nc.vector.tensor_mul(e[:, jc], e[:, jc],
                                                             em[:, g, :])
                                eps_pair.append(e)
                            for hh in range(2):
                                nc.tensor.matmul(
                                    oas[hh][:, lo:], vaug[:, b, :],
                                    eps_pair[hh][:, :n], start=(bi == 0),
                                    stop=(bi == len(blocks) - 1))
                        for hh in range(2):
                            oa = oas[hh]
                            ou = t3.tile([64, CH], F32, tag='ou')
                            nc.vector.tensor_copy(ou[:], oa[0:HD, :])
                            # 1/Z on ScalarE via exp(-ln Z); DVE recip is slow
                            zln = t3.tile([1, CH], F32, tag='zln')
                            nc.scalar.activation(zln[:], oa[64:65, :], func=Ln)
                            rz = t3.tile([1, CH], F32R, tag='rz')
                            nc.scalar.activation(rz[:], zln[:], func=Exp,
                                                 scale=-1.0)
                            zb = zp.tile([64, CH], F32, tag='zb')
                            nc.tensor.matmul(zb[:], ones_row[0:1, :], rz[:],
                                             start=True, stop=True)
                            nc.vector.tensor_mul(
                                ot[64 * hh:64 * hh + HD, hp, cs], ou[:], zb[:])

            # ---------------- phase 3: output projection ---------------------
            yT_r = yT_d.rearrange('(o p) s -> p o s', p=P)
            with tc.tile_pool(name='t4', bufs=4) as t4, \
                 tc.tile_pool(name='w3', bufs=1) as w3, \
                 tc.tile_pool(name='yp', bufs=4, space='PSUM') as yp:
                wo = w3.tile([P, 2, S], F32R)
                nc.sync.dma_start(wo[:], woT_d.rearrange('(o p) m -> p o m', p=P))
                for blk in range(NKT):
                    for c in range(NCH):
                        cs = slice(c * CH, (c + 1) * CH)
                        ps = yp.tile([P, CH], F32, tag='y')
                        for kt in range(2):
                            nc.tensor.matmul(
                                ps[:], wo[:, kt, blk * P:(blk + 1) * P],
                                ot[:, kt, cs], start=(kt == 0), stop=(kt == 1))
                        ys = t4.tile([P, CH], F32, tag='ys')
                        nc.any.tensor_copy(ys[:], ps[:])
                        nc.gpsimd.dma_start(yT_r[:, blk, cs], ys[:])
                nc.gpsimd.dma_start(kh_d[:], kst.bitcast(F32)[0:HD, :])
                nc.gpsimd.dma_start(
                    vh_d.rearrange('(o p) h -> p o h', p=P),
                    vaug.bitcast(F32)[:, :, 0:HD])
    nc.compile()
    return nc


_cache = {}


def kernel(hidden_states, freqs_cos, freqs_sin, atten_mask, wq, wk, wv, wo,
           q_norm_w, k_norm_w):
    from concourse.bass_utils import run_bass_kernel_spmd

    hs = np.asarray(hidden_states, dtype=np.float32)
    fc = np.asarray(freqs_cos, dtype=np.float32)
    fs = np.asarray(freqs_sin, dtype=np.float32)
    am = np.asarray(atten_mask, dtype=np.float32)
    wq = np.asarray(wq, dtype=np.float32)
    wk = np.asarray(wk, dtype=np.float32)
    wv = np.asarray(wv, dtype=np.float32)
    wo = np.asarray(wo, dtype=np.float32)
    qw = np.asarray(q_norm_w, dtype=np.float32)
    kw = np.asarray(k_norm_w, dtype=np.float32)

    grid, tiles = _classify_mask(am[0, 0])
    key = (grid.tobytes(), len(tiles))
    if key not in _cache:
        _cache[key] = _build(grid, len(tiles))
    nc = _cache[key]

    em = np.stack(tiles) if tiles else np.zeros((1, P, P), np.float32)
    xT = np.ascontiguousarray(hs[0].T)

    # rope tables in transposed + deinterleaved layout
    cosT = np.ascontiguousarray(fc.T)  # [32, S]
    sinT = np.ascontiguousarray(fs.T)
    qw_p = qw[PERM]
    kw_p = kw[PERM]

    c64 = np.concatenate([cosT, cosT], 0)           # [64, S]
    s64 = np.concatenate([sinT, -sinT], 0)          # indexed by SOURCE row
    cq_t, sq_t = np.tile(c64, (2, 1)), np.tile(s64, (2, 1))
    ck_t, sk_t = c64, s64

    # norm weights as lhsT for the rsqrt-broadcast matmuls:
    # q: lhsT = wrow[0:2, 0:128] (K=2: row 0 scales head-even rows, row 1 odd)
    # k: lhsT = wrow[0:1, 128:192]
    wrow = np.zeros((2, 2 * P), dtype=np.float32)
    wrow[0, 0:64] = qw_p
    wrow[1, 64:128] = qw_p
    wrow[0, P:P + 64] = kw_p

    wq4 = wq.reshape(NH, HD, D)
    wk8 = wk.reshape(NKV, HD, D)
    wv8 = wv.reshape(NKV, HD, D)

    in_maps = []
    for c in range(NCORES):
        wq_c = wq4[HPC * c:HPC * (c + 1)][:, PERM, :]      # [4, 64, D]
        wqT = np.ascontiguousarray(
            wq_c.reshape(HPC * HD, D).T)                    # [D, 256]
        wkv = np.concatenate([wk8[c][PERM], wv8[c]], 0)     # [128, D]
        wkvT = np.ascontiguousarray(wkv.T)
        woT = np.ascontiguousarray(wo[:, HPC * HD * c:HPC * HD * (c + 1)].T)
        in_maps.append(dict(xT=xT, wqT=wqT, wkvT=wkvT, woT=woT,
                            cq=np.ascontiguousarray(cq_t),
                            sq=np.ascontiguousarray(sq_t),
                            ck=np.ascontiguousarray(ck_t),
                            sk=np.ascontiguousarray(sk_t), em=em, wrow=wrow))

    global _last_in_maps
    _last_in_maps = in_maps
    res = run_bass_kernel_spmd(nc, in_maps, core_ids=list(range(NCORES)))
    outs = res.results

    y = np.zeros((S, D), dtype=np.float32)
    khT = np.empty((B, NH, HD, S), dtype=np.float32)
    vh = np.empty((B, NH, S, HD), dtype=np.float32)
    inv = np.argsort(PERM)
    for c in range(NCORES):
        y += outs[c]['yT'].T
        k_nat = outs[c]['kh'][inv]          # un-permute rows
        for h in range(HPC):
            khT[0, HPC * c + h] = k_nat
            vh[0, HPC * c + h] = outs[c]['vh']
    return y.reshape(B, S, D), khT, vh
